# revision 5
# baseline (speedup 1.0000x reference)
"""DynamicRelationshipExtractor Trainium2 kernel (8 NeuronCores, batch-sharded).

Sharding: core k handles batches [8k, 8k+8) x all 2016 pairs (16128 tokens).
All cores run ONE SPMD program; per-core data differences enter via inputs.
Pair indices (idx_i/idx_j) are baked into the program's access patterns at
trace time (they are identical across cores under batch sharding).

Algebraic restructuring vs the reference:
- cosine feature = An*Bn with An = x/(||x||+eps) precomputed per column (host).
- divide features use xrec = 1/(x+eps) precomputed per column (host).
- sub_ba feature = -(A-B): its first-layer matmul reuses the A-B feature with
  negated weights, so only 6 feature tensors are materialized on device.
- The fusion first layer is folded through the op-MLP second layers:
  u = sum_k h_k @ M_k + presence-contrib + c0, with M_k = W2_k @ fus_W1[blk k].
- The presence MLP collapses to a 4-row table (flags are one-hot); it enters u
  via a small-K matmul against host-built one-hot rows; fus_b1/b2_k/pres_b2
  fold into a constant row c0 paired with a ones row; fus_b2 enters the last
  matmul through the same ones row.

Device layout: columns are (c-major, batch-inner): col = c*8 + b_local, so a
j-run of pairs for fixed i is a contiguous 2D access pattern and the A operand
is a step-0 broadcast. Output is [128=d, token] per core, token = p*8+b_local,
reassembled on host.
"""
import sys
import numpy as np
import ml_dtypes
from contextlib import ExitStack

if '/opt/trn_rl_repo' not in sys.path:
    sys.path.insert(0, '/opt/trn_rl_repo')

import concourse.bass as bass
import concourse.tile as tile
from concourse import mybir
from concourse.bass_utils import run_bass_kernel_spmd

bf16 = mybir.dt.bfloat16
f32 = mybir.dt.float32
AF = mybir.ActivationFunctionType
ALU = mybir.AluOpType
BF = ml_dtypes.bfloat16

D = 128
NCOLS = 64
B = 64
P_TOT = 2016
EPS = 1e-6
MLP_IDX = [0, 1, 2, 3, 4, 4, 5, 5]
NCORES = 8
BC = B // NCORES            # batches per core = 8
NTOK = P_TOT * BC           # tokens per core = 16128
PAIRS_PER_TILE = 64         # 512 tokens per tile
NCOL_DEV = NCOLS * BC       # 512 device columns of x panels

_CACHE = {}


def _legalize_multi_waits(nc, max_waits=1):
    n_new = 0
    for f in nc.m.functions:
        for bb in f.blocks:
            new_list = []
            for inst in bb.instructions:
                si = inst.sync_info
                ow = list(si.on_wait) if si and si.on_wait else []
                if len(ow) > max_waits:
                    keep = ow[:max_waits]
                    for w in ow[max_waits:]:
                        nop = mybir.InstNoOp(name=f"I-mwsplit-{n_new}", ins=[], outs=[])
                        nop.engine = inst.engine
                        nop.sync_info = mybir.SyncInfo(on_wait=[w], on_update=[])
                        new_list.append(nop)
                        n_new += 1
                    inst.sync_info = mybir.SyncInfo(
                        on_wait=keep,
                        on_update=list(si.on_update) if si.on_update else [],
                    )
                new_list.append(inst)
            bb.instructions = new_list
    return n_new


def _runs(idx_i, idx_j):
    """Tile the 2016 pairs into PAIRS_PER_TILE tiles; within each tile, split
    into (i, j0, L) runs with constant i and consecutive j."""
    tiles = []
    p = 0
    while p < P_TOT:
        p_end = min(p + PAIRS_PER_TILE, P_TOT)
        runs = []
        q = p
        while q < p_end:
            i0, j0 = int(idx_i[q]), int(idx_j[q])
            n = 1
            while (q + n < p_end and int(idx_i[q + n]) == i0
                   and int(idx_j[q + n]) == j0 + n):
                n += 1
            runs.append((q - p, i0, j0, n))
            q += n
        tiles.append((p, p_end - p, runs))
        p = p_end
    return tiles


def _build_program(tiles):
    nc = bass.Bass()
    xT_in = nc.declare_dram_parameter("xT", [D, NCOL_DEV], bf16, isOutput=False)
    xn_in = nc.declare_dram_parameter("xn", [D, NCOL_DEV], bf16, isOutput=False)
    xr_in = nc.declare_dram_parameter("xr", [D, NCOL_DEV], bf16, isOutput=False)
    oh_in = nc.declare_dram_parameter("oh", [5, NTOK], bf16, isOutput=False)
    w1_in = nc.declare_dram_parameter("w1", [D, 8 * 64], bf16, isOutput=False)
    mf_in = nc.declare_dram_parameter("mf", [D, 4 * 256], bf16, isOutput=False)
    pt_in = nc.declare_dram_parameter("pt", [5, 256], bf16, isOutput=False)
    w2_in = nc.declare_dram_parameter("w2", [D, 256], bf16, isOutput=False)
    b2_in = nc.declare_dram_parameter("b2", [1, D], bf16, isOutput=False)
    bc_in = nc.declare_dram_parameter("bc", [D, 4], f32, isOutput=False)
    out = nc.declare_dram_parameter("out", [D, NTOK], f32, isOutput=True)

    with tile.TileContext(nc) as tc:
        ctx = ExitStack()
        const = ctx.enter_context(tc.tile_pool(name="const", bufs=1))
        featp = ctx.enter_context(tc.tile_pool(name="featp", bufs=2))
        chunkp = ctx.enter_context(tc.tile_pool(name="chunkp", bufs=2))
        outp_p = ctx.enter_context(tc.tile_pool(name="outp_p", bufs=2))
        psz = ctx.enter_context(tc.tile_pool(name="psz", bufs=1, space="PSUM"))
        psu = ctx.enter_context(tc.tile_pool(name="psu", bufs=1, space="PSUM"))

        # ---- constants in SBUF ----
        xT = const.tile([D, NCOL_DEV], bf16)
        nc.sync.dma_start(xT[:], xT_in[:])
        xn = const.tile([D, NCOL_DEV], bf16)
        nc.sync.dma_start(xn[:], xn_in[:])
        xr = const.tile([D, NCOL_DEV], bf16)
        nc.sync.dma_start(xr[:], xr_in[:])
        oh = const.tile([5, NTOK], bf16)
        nc.sync.dma_start(oh[:], oh_in[:])
        w1 = const.tile([D, 8 * 64], bf16)
        nc.sync.dma_start(w1[:], w1_in[:])
        mf = const.tile([D, 4 * 256], bf16)
        nc.sync.dma_start(mf[:], mf_in[:])
        pt = const.tile([5, 256], bf16)
        nc.sync.dma_start(pt[:], pt_in[:])
        w2 = const.tile([D, 256], bf16)
        nc.sync.dma_start(w2[:], w2_in[:])
        b2r = const.tile([1, D], bf16)
        nc.sync.dma_start(b2r[:], b2_in[:])
        bc = const.tile([D, 4], f32)
        nc.sync.dma_start(bc[:], bc_in[:])

        xTv = xT[:].rearrange("p (c b) -> p c b", b=BC)
        xnv = xn[:].rearrange("p (c b) -> p c b", b=BC)
        xrv = xr[:].rearrange("p (c b) -> p c b", b=BC)

        # stage-1 weight order: mult, cos, abs, divab, divba, add, sub, subba
        W = [w1[:, 64 * k:64 * (k + 1)] for k in range(8)]

        for (p0, npair, runs) in tiles:
            NT = npair * BC  # tokens this tile
            t0 = p0 * BC

            # ---- feature tiles (bf16) ----
            f_mult = featp.tile([D, NT], bf16, tag="f_mult", name="f_mult")
            f_cos = featp.tile([D, NT], bf16, tag="f_cos", name="f_cos")
            f_diff = featp.tile([D, NT], bf16, tag="f_diff", name="f_diff")
            f_absd = featp.tile([D, NT], bf16, tag="f_absd", name="f_absd")
            f_dab = featp.tile([D, NT], bf16, tag="f_dab", name="f_dab")
            f_dba = featp.tile([D, NT], bf16, tag="f_dba", name="f_dba")
            f_add = featp.tile([D, NT], bf16, tag="f_add", name="f_add")
            views = {
                k: t[:].rearrange("p (l b) -> p l b", b=BC)
                for k, t in (("mult", f_mult), ("cos", f_cos), ("diff", f_diff),
                             ("dab", f_dab), ("dba", f_dba), ("add", f_add))
            }
            for (po, i0, j0, L) in runs:
                sl = (slice(None), slice(po, po + L), slice(None))
                Ax = xTv[:, i0, :][:, None, :].broadcast_to((D, L, BC))
                Bx = xTv[:, j0:j0 + L, :]
                An = xnv[:, i0, :][:, None, :].broadcast_to((D, L, BC))
                Bn = xnv[:, j0:j0 + L, :]
                Ar = xrv[:, i0, :][:, None, :].broadcast_to((D, L, BC))
                Br = xrv[:, j0:j0 + L, :]
                nc.gpsimd.tensor_mul(views["mult"][sl], Ax, Bx)
                nc.vector.tensor_mul(views["cos"][sl], An, Bn)
                nc.vector.tensor_sub(views["diff"][sl], Ax, Bx)
                nc.vector.tensor_mul(views["dab"][sl], Ax, Br)
                nc.gpsimd.tensor_mul(views["dba"][sl], Bx, Ar)
                nc.gpsimd.tensor_add(views["add"][sl], Ax, Bx)
            # |A-B| on the scalar engine (ACT Abs table)
            nc.scalar.activation(f_absd[:], f_diff[:], AF.Abs)

            # ---- stage 1 matmuls: 4 col-packed psum banks ----
            zA = psz.tile([D, NT], f32, tag="zA", name="zA")
            zB = psz.tile([D, NT], f32, tag="zB", name="zB")
            zC = psz.tile([D, NT], f32, tag="zC", name="zC")
            zD = psz.tile([D, NT], f32, tag="zD", name="zD")
            nc.tensor.matmul(zA[0:64, :], W[0], f_mult[:], start=True, stop=True,
                             tile_position=(0, 0))
            nc.tensor.matmul(zA[64:128, :], W[1], f_cos[:], start=True, stop=True,
                             tile_position=(0, 64))
            nc.tensor.matmul(zB[0:64, :], W[2], f_absd[:], start=True, stop=True,
                             tile_position=(0, 0))
            nc.tensor.matmul(zB[64:128, :], W[3], f_dab[:], start=True, stop=True,
                             tile_position=(0, 64))
            nc.tensor.matmul(zC[0:64, :], W[4], f_dba[:], start=True, stop=True,
                             tile_position=(0, 0))
            nc.tensor.matmul(zC[64:128, :], W[5], f_add[:], start=True, stop=True,
                             tile_position=(0, 64))
            nc.tensor.matmul(zD[0:64, :], W[6], f_diff[:], start=True, stop=True,
                             tile_position=(0, 0))
            nc.tensor.matmul(zD[64:128, :], W[7], f_diff[:], start=True, stop=True,
                             tile_position=(0, 64))

            # ---- relu+bias evictions -> bf16 chunks ----
            chunks = []
            for ci, z in enumerate((zA, zB, zC, zD)):
                ch = chunkp.tile([D, NT], bf16, tag=f"ch{ci}", name=f"ch{ci}")
                nc.scalar.activation(ch[:], z[:], AF.Relu, bias=bc[:, ci:ci + 1])
                chunks.append(ch)

            # ---- stage 2: u = sum_c chunk_c @ Mf_c + presence table ----
            u_lo = psu.tile([D, NT], f32, tag="u_lo", name="u_lo")
            u_hi = psu.tile([D, NT], f32, tag="u_hi", name="u_hi")
            for ci in range(4):
                nc.tensor.matmul(u_lo[:], mf[:, ci * 256:ci * 256 + 128],
                                 chunks[ci][:], start=(ci == 0), stop=False)
                nc.tensor.matmul(u_hi[:], mf[:, ci * 256 + 128:ci * 256 + 256],
                                 chunks[ci][:], start=(ci == 0), stop=False)
            ohs = oh[:, t0:t0 + NT]
            nc.tensor.matmul(u_lo[:], pt[:, 0:128], ohs, start=False, stop=True)
            nc.tensor.matmul(u_hi[:], pt[:, 128:256], ohs, start=False, stop=True)

            # ---- relu(u) -> bf16 ----
            r_lo = chunkp.tile([D, NT], bf16, tag="r_lo", name="r_lo")
            r_hi = chunkp.tile([D, NT], bf16, tag="r_hi", name="r_hi")
            nc.vector.tensor_scalar_max(r_lo[:], u_lo[:], 0.0)
            nc.vector.tensor_scalar_max(r_hi[:], u_hi[:], 0.0)

            # ---- stage 3: out = r @ fus_W2 + fus_b2 (b2 via ones row) ----
            op = psu.tile([D, NT], f32, tag="op", name="op")
            nc.tensor.matmul(op[:], b2r[:], oh[0:1, t0:t0 + NT],
                             start=True, stop=False)
            nc.tensor.matmul(op[:], w2[:, 0:128], r_lo[:], start=False, stop=False)
            nc.tensor.matmul(op[:], w2[:, 128:256], r_hi[:], start=False, stop=True)

            osb = outp_p.tile([D, NT], f32, tag="osb", name="osb")
            nc.scalar.copy(osb[:], op[:])
            nc.sync.dma_start(out[:, t0:t0 + NT], osb[:])
        ctx.close()

    _legalize_multi_waits(nc)
    return nc


def _prep_host(x, presence, idx_i, idx_j,
               ops_W1, ops_b1, ops_W2, ops_b2,
               pres_W1, pres_b1, pres_W2, pres_b2,
               fus_W1, fus_b1, fus_W2, fus_b2):
    x = np.asarray(x, np.float32)
    ops_W1 = np.asarray(ops_W1, np.float32)
    ops_b1 = np.asarray(ops_b1, np.float32)
    ops_W2 = np.asarray(ops_W2, np.float32)
    ops_b2 = np.asarray(ops_b2, np.float32)
    fus_W1 = np.asarray(fus_W1, np.float32)
    fus_b1 = np.asarray(fus_b1, np.float32)
    fus_W2 = np.asarray(fus_W2, np.float32)
    fus_b2 = np.asarray(fus_b2, np.float32)
    pres_W1 = np.asarray(pres_W1, np.float32)
    pres_b1 = np.asarray(pres_b1, np.float32)
    pres_W2 = np.asarray(pres_W2, np.float32)
    pres_b2 = np.asarray(pres_b2, np.float32)

    # per-core x panels, cols (c-major, b-inner)
    xTs, xns, xrs = [], [], []
    norms = np.linalg.norm(x, axis=2)                    # [B, NCOLS]
    xn_full = x / (norms[:, :, None] + EPS)
    xr_full = 1.0 / (x + EPS)
    for k in range(NCORES):
        bs = slice(k * BC, (k + 1) * BC)
        # [BC, NCOLS, D] -> [D, NCOLS, BC] -> [D, NCOLS*BC]
        def panel(a):
            return np.ascontiguousarray(
                np.transpose(a[bs], (2, 1, 0)).reshape(D, NCOL_DEV)).astype(BF)
        xTs.append(panel(x))
        xns.append(panel(xn_full))
        xrs.append(panel(xr_full))

    # one-hot rows [ones, t0, t1, t2, t3] per core; token = p*BC + b_local
    pa = np.asarray(presence)[:, idx_i].astype(np.int64)   # [B, P]
    pb = np.asarray(presence)[:, idx_j].astype(np.int64)
    tsel = 2 * pa + pb
    ohs = []
    for k in range(NCORES):
        bs = slice(k * BC, (k + 1) * BC)
        tloc = np.transpose(tsel[bs], (1, 0)).reshape(NTOK)  # p-major, b-inner
        o = np.zeros((5, NTOK), np.float32)
        o[0] = 1.0
        for tt in range(4):
            o[1 + tt] = (tloc == tt)
        ohs.append(o.astype(BF))

    # stage-1 weights: mult, cos, abs, divab, divba, add, sub, subba(-W1_4)
    Wcat = np.concatenate([
        ops_W1[0], ops_W1[2], ops_W1[3], ops_W1[5],
        ops_W1[5], ops_W1[1], ops_W1[4], -ops_W1[4]], axis=1)  # [128, 512]

    # chunk h-order: [mult,cos | absd,divab | divba,add | sub,subba]
    feat_fusblk = [0, 2, 3, 6, 7, 1, 4, 5]   # fus_W1 block per h slot
    feat_w2 = [MLP_IDX[i] for i in feat_fusblk]
    Mf = np.zeros((512, 256), np.float32)
    c0 = fus_b1.copy()
    for s in range(8):
        blk = fus_W1[feat_fusblk[s] * D:(feat_fusblk[s] + 1) * D]  # [128,256]
        Mf[s * 64:(s + 1) * 64] = ops_W2[feat_w2[s]] @ blk
        c0 += ops_b2[feat_w2[s]] @ blk
    # device layout [128, 4*256]: block c at cols [c*256, (c+1)*256)
    Mf_dev = np.zeros((128, 4 * 256), np.float32)
    for c in range(4):
        Mf_dev[:, c * 256:(c + 1) * 256] = Mf[c * 128:(c + 1) * 128]

    # presence table + c0; rows [c0, t0..t3] matching onehot rows
    fus_blk_p = fus_W1[8 * D:]
    Mp = pres_W2 @ fus_blk_p
    ptabc = np.zeros((5, 256), np.float32)
    ptabc[0] = c0 + pres_b2 @ fus_blk_p
    for tt in range(4):
        hp = np.maximum(pres_W1[tt] + pres_b1, 0.0)
        ptabc[1 + tt] = hp @ Mp

    # chunk relu biases: [b1_mult;b1_cos], [b1_abs;b1_divab],
    #                    [b1_divba;b1_add], [b1_sub;b1_subba]
    bcat = np.stack([
        np.concatenate([ops_b1[0], ops_b1[2]]),
        np.concatenate([ops_b1[3], ops_b1[5]]),
        np.concatenate([ops_b1[5], ops_b1[1]]),
        np.concatenate([ops_b1[4], ops_b1[4]]),
    ], axis=1).astype(np.float32)                         # [128, 4]

    w2_dev = np.concatenate([fus_W2[0:128], fus_W2[128:256]], axis=1)  # [128,256]

    shared = dict(
        w1=np.ascontiguousarray(Wcat).astype(BF),
        mf=np.ascontiguousarray(Mf_dev).astype(BF),
        pt=np.ascontiguousarray(ptabc).astype(BF),
        w2=np.ascontiguousarray(w2_dev).astype(BF),
        b2=np.ascontiguousarray(fus_b2[None, :]).astype(BF),
        bc=np.ascontiguousarray(bcat),
    )
    in_maps = []
    for k in range(NCORES):
        m = dict(shared)
        m["xT"] = xTs[k]
        m["xn"] = xns[k]
        m["xr"] = xrs[k]
        m["oh"] = ohs[k]
        in_maps.append(m)
    return in_maps


def kernel(x, presence, idx_i, idx_j,
           ops_W1, ops_b1, ops_W2, ops_b2,
           pres_W1, pres_b1, pres_W2, pres_b2,
           fus_W1, fus_b1, fus_W2, fus_b2):
    idx_i = np.asarray(idx_i)
    idx_j = np.asarray(idx_j)
    key = (idx_i.tobytes(), idx_j.tobytes())
    if key not in _CACHE:
        _CACHE.clear()
        _CACHE[key] = _build_program(_runs(idx_i, idx_j))
    nc = _CACHE[key]

    in_maps = _prep_host(x, presence, idx_i, idx_j,
                         ops_W1, ops_b1, ops_W2, ops_b2,
                         pres_W1, pres_b1, pres_W2, pres_b2,
                         fus_W1, fus_b1, fus_W2, fus_b2)
    res = run_bass_kernel_spmd(nc, in_maps, core_ids=list(range(NCORES)))
    kernel._last_results = res

    # reassemble: out_full[b, p, d] = core_out[b//BC][d, p*BC + b%BC]
    out = np.empty((B, P_TOT, D), np.float32)
    for k in range(NCORES):
        co = res.results[k]["out"]                  # [D, NTOK]
        co = co.reshape(D, P_TOT, BC)               # [d, p, b_local]
        out[k * BC:(k + 1) * BC] = np.transpose(co, (2, 1, 0))
    return out.astype(np.asarray(x).dtype)


# revision 6
# speedup vs baseline: 1.0344x; 1.0344x over previous
"""DynamicRelationshipExtractor Trainium2 kernel (8 NeuronCores, batch-sharded).

Sharding: core k handles batches [8k, 8k+8) x all 2016 pairs (16128 tokens).
All cores run ONE SPMD program; per-core data differences enter via inputs.
Pair indices (idx_i/idx_j) are baked into the program's access patterns at
trace time (they are identical across cores under batch sharding).

Algebraic restructuring vs the reference:
- cosine feature = An*Bn with An = x/(||x||+eps) precomputed per column (host).
- divide features use xrec = 1/(x+eps) precomputed per column (host).
- sub_ba feature = -(A-B): its first-layer matmul reuses the A-B feature with
  negated weights, so only 6 feature tensors are materialized on device.
- The fusion first layer is folded through the op-MLP second layers:
  u = sum_k h_k @ M_k + presence-contrib + c0, with M_k = W2_k @ fus_W1[blk k].
- The presence MLP collapses to a 4-row table (flags are one-hot); it enters u
  via a small-K matmul against host-built one-hot rows; fus_b1/b2_k/pres_b2
  fold into a constant row c0 paired with a ones row; fus_b2 enters the last
  matmul through the same ones row.

Device layout: columns are (c-major, batch-inner): col = c*8 + b_local, so a
j-run of pairs for fixed i is a contiguous 2D access pattern and the A operand
is a step-0 broadcast. Output is [128=d, token] per core, token = p*8+b_local,
reassembled on host.
"""
import sys
import numpy as np
import ml_dtypes
from contextlib import ExitStack

if '/opt/trn_rl_repo' not in sys.path:
    sys.path.insert(0, '/opt/trn_rl_repo')

import concourse.bass as bass
import concourse.tile as tile
from concourse import mybir
from concourse.bass_utils import run_bass_kernel_spmd

bf16 = mybir.dt.bfloat16
f32 = mybir.dt.float32
AF = mybir.ActivationFunctionType
ALU = mybir.AluOpType
BF = ml_dtypes.bfloat16

D = 128
NCOLS = 64
B = 64
P_TOT = 2016
EPS = 1e-6
MLP_IDX = [0, 1, 2, 3, 4, 4, 5, 5]
NCORES = 8
BC = B // NCORES            # batches per core = 8
NTOK = P_TOT * BC           # tokens per core = 16128
PAIRS_PER_TILE = 64         # 512 tokens per tile
NCOL_DEV = NCOLS * BC       # 512 device columns of x panels

_CACHE = {}


def _legalize_multi_waits(nc, max_waits=1):
    n_new = 0
    for f in nc.m.functions:
        for bb in f.blocks:
            new_list = []
            for inst in bb.instructions:
                si = inst.sync_info
                ow = list(si.on_wait) if si and si.on_wait else []
                if len(ow) > max_waits:
                    keep = ow[:max_waits]
                    for w in ow[max_waits:]:
                        nop = mybir.InstNoOp(name=f"I-mwsplit-{n_new}", ins=[], outs=[])
                        nop.engine = inst.engine
                        nop.sync_info = mybir.SyncInfo(on_wait=[w], on_update=[])
                        new_list.append(nop)
                        n_new += 1
                    inst.sync_info = mybir.SyncInfo(
                        on_wait=keep,
                        on_update=list(si.on_update) if si.on_update else [],
                    )
                new_list.append(inst)
            bb.instructions = new_list
    return n_new


def _runs(idx_i, idx_j):
    """Tile the 2016 pairs into PAIRS_PER_TILE tiles; within each tile, split
    into (i, j0, L) runs with constant i and consecutive j."""
    tiles = []
    p = 0
    while p < P_TOT:
        p_end = min(p + PAIRS_PER_TILE, P_TOT)
        runs = []
        q = p
        while q < p_end:
            i0, j0 = int(idx_i[q]), int(idx_j[q])
            n = 1
            while (q + n < p_end and int(idx_i[q + n]) == i0
                   and int(idx_j[q + n]) == j0 + n):
                n += 1
            runs.append((q - p, i0, j0, n))
            q += n
        tiles.append((p, p_end - p, runs))
        p = p_end
    return tiles


def _build_program(tiles):
    nc = bass.Bass()
    xT_in = nc.declare_dram_parameter("xT", [D, NCOL_DEV], bf16, isOutput=False)
    xn_in = nc.declare_dram_parameter("xn", [D, NCOL_DEV], bf16, isOutput=False)
    xr_in = nc.declare_dram_parameter("xr", [D, NCOL_DEV], bf16, isOutput=False)
    oh_in = nc.declare_dram_parameter("oh", [5, NTOK], bf16, isOutput=False)
    w1_in = nc.declare_dram_parameter("w1", [D, 8 * 64], bf16, isOutput=False)
    mf_in = nc.declare_dram_parameter("mf", [D, 4 * 256], bf16, isOutput=False)
    pt_in = nc.declare_dram_parameter("pt", [5, 256], bf16, isOutput=False)
    w2_in = nc.declare_dram_parameter("w2", [D, 256], bf16, isOutput=False)
    bc_in = nc.declare_dram_parameter("bc", [D, 5], f32, isOutput=False)
    out = nc.declare_dram_parameter("out", [D, NTOK], f32, isOutput=True)

    with tile.TileContext(nc) as tc:
        ctx = ExitStack()
        const = ctx.enter_context(tc.tile_pool(name="const", bufs=1))
        featp = ctx.enter_context(tc.tile_pool(name="featp", bufs=2))
        chunkp = ctx.enter_context(tc.tile_pool(name="chunkp", bufs=2))
        outp_p = ctx.enter_context(tc.tile_pool(name="outp_p", bufs=2))
        psz = ctx.enter_context(tc.tile_pool(name="psz", bufs=1, space="PSUM"))
        psu = ctx.enter_context(tc.tile_pool(name="psu", bufs=1, space="PSUM"))

        # ---- constants in SBUF ----
        xT = const.tile([D, NCOL_DEV], bf16)
        nc.sync.dma_start(xT[:], xT_in[:])
        xn = const.tile([D, NCOL_DEV], bf16)
        nc.sync.dma_start(xn[:], xn_in[:])
        xr = const.tile([D, NCOL_DEV], bf16)
        nc.sync.dma_start(xr[:], xr_in[:])
        oh = const.tile([5, NTOK], bf16)
        nc.sync.dma_start(oh[:], oh_in[:])
        w1 = const.tile([D, 8 * 64], bf16)
        nc.sync.dma_start(w1[:], w1_in[:])
        mf = const.tile([D, 4 * 256], bf16)
        nc.sync.dma_start(mf[:], mf_in[:])
        pt = const.tile([5, 256], bf16)
        nc.sync.dma_start(pt[:], pt_in[:])
        w2 = const.tile([D, 256], bf16)
        nc.sync.dma_start(w2[:], w2_in[:])
        bc = const.tile([D, 5], f32)
        nc.sync.dma_start(bc[:], bc_in[:])

        xTv = xT[:].rearrange("p (c b) -> p c b", b=BC)
        xnv = xn[:].rearrange("p (c b) -> p c b", b=BC)
        xrv = xr[:].rearrange("p (c b) -> p c b", b=BC)

        # stage-1 weight order: mult, cos, abs, divab, divba, add, sub, subba
        W = [w1[:, 64 * k:64 * (k + 1)] for k in range(8)]

        for (p0, npair, runs) in tiles:
            NT = npair * BC  # tokens this tile
            t0 = p0 * BC

            # ---- feature tiles (bf16) ----
            f_mult = featp.tile([D, NT], bf16, tag="f_mult", name="f_mult")
            f_cos = featp.tile([D, NT], bf16, tag="f_cos", name="f_cos")
            f_diff = featp.tile([D, NT], bf16, tag="f_diff", name="f_diff")
            f_absd = featp.tile([D, NT], bf16, tag="f_absd", name="f_absd")
            f_dab = featp.tile([D, NT], bf16, tag="f_dab", name="f_dab")
            f_dba = featp.tile([D, NT], bf16, tag="f_dba", name="f_dba")
            f_add = featp.tile([D, NT], bf16, tag="f_add", name="f_add")
            views = {
                k: t[:].rearrange("p (l b) -> p l b", b=BC)
                for k, t in (("mult", f_mult), ("cos", f_cos), ("diff", f_diff),
                             ("dab", f_dab), ("dba", f_dba), ("add", f_add))
            }
            for (po, i0, j0, L) in runs:
                sl = (slice(None), slice(po, po + L), slice(None))
                Ax = xTv[:, i0, :][:, None, :].broadcast_to((D, L, BC))
                Bx = xTv[:, j0:j0 + L, :]
                An = xnv[:, i0, :][:, None, :].broadcast_to((D, L, BC))
                Bn = xnv[:, j0:j0 + L, :]
                Ar = xrv[:, i0, :][:, None, :].broadcast_to((D, L, BC))
                Br = xrv[:, j0:j0 + L, :]
                nc.gpsimd.tensor_mul(views["mult"][sl], Ax, Bx)
                nc.vector.tensor_mul(views["cos"][sl], An, Bn)
                nc.vector.tensor_sub(views["diff"][sl], Ax, Bx)
                nc.vector.tensor_mul(views["dab"][sl], Ax, Br)
                nc.gpsimd.tensor_mul(views["dba"][sl], Bx, Ar)
                nc.gpsimd.tensor_add(views["add"][sl], Ax, Bx)
            # |A-B| on the scalar engine (ACT Abs table)
            nc.scalar.activation(f_absd[:], f_diff[:], AF.Abs)

            # ---- stage 1 matmuls: 4 col-packed psum banks ----
            zA = psz.tile([D, NT], f32, tag="zA", name="zA")
            zB = psz.tile([D, NT], f32, tag="zB", name="zB")
            zC = psz.tile([D, NT], f32, tag="zC", name="zC")
            zD = psz.tile([D, NT], f32, tag="zD", name="zD")
            nc.tensor.matmul(zA[0:64, :], W[0], f_mult[:], start=True, stop=True,
                             tile_position=(0, 0))
            nc.tensor.matmul(zA[64:128, :], W[1], f_cos[:], start=True, stop=True,
                             tile_position=(0, 64))
            nc.tensor.matmul(zB[0:64, :], W[2], f_absd[:], start=True, stop=True,
                             tile_position=(0, 0))
            nc.tensor.matmul(zB[64:128, :], W[3], f_dab[:], start=True, stop=True,
                             tile_position=(0, 64))
            nc.tensor.matmul(zC[0:64, :], W[4], f_dba[:], start=True, stop=True,
                             tile_position=(0, 0))
            nc.tensor.matmul(zC[64:128, :], W[5], f_add[:], start=True, stop=True,
                             tile_position=(0, 64))
            nc.tensor.matmul(zD[0:64, :], W[6], f_diff[:], start=True, stop=True,
                             tile_position=(0, 0))
            nc.tensor.matmul(zD[64:128, :], W[7], f_diff[:], start=True, stop=True,
                             tile_position=(0, 64))

            # ---- relu+bias evictions -> bf16 chunks ----
            chunks = []
            for ci, z in enumerate((zA, zB, zC, zD)):
                ch = chunkp.tile([D, NT], bf16, tag=f"ch{ci}", name=f"ch{ci}")
                nc.scalar.activation(ch[:], z[:], AF.Relu, bias=bc[:, ci:ci + 1])
                chunks.append(ch)

            # ---- stage 2: u = sum_c chunk_c @ Mf_c + presence table ----
            u_lo = psu.tile([D, NT], f32, tag="u_lo", name="u_lo")
            u_hi = psu.tile([D, NT], f32, tag="u_hi", name="u_hi")
            for ci in range(4):
                nc.tensor.matmul(u_lo[:], mf[:, ci * 256:ci * 256 + 128],
                                 chunks[ci][:], start=(ci == 0), stop=False)
                nc.tensor.matmul(u_hi[:], mf[:, ci * 256 + 128:ci * 256 + 256],
                                 chunks[ci][:], start=(ci == 0), stop=False)
            ohs = oh[:, t0:t0 + NT]
            nc.tensor.matmul(u_lo[:], pt[:, 0:128], ohs, start=False, stop=True)
            nc.tensor.matmul(u_hi[:], pt[:, 128:256], ohs, start=False, stop=True)

            # ---- relu(u) -> bf16 ----
            r_lo = chunkp.tile([D, NT], bf16, tag="r_lo", name="r_lo")
            r_hi = chunkp.tile([D, NT], bf16, tag="r_hi", name="r_hi")
            nc.vector.tensor_scalar_max(r_lo[:], u_lo[:], 0.0)
            nc.vector.tensor_scalar_max(r_hi[:], u_hi[:], 0.0)

            # ---- stage 3: out = r @ fus_W2 + fus_b2 (b2 via ones row) ----
            op = psu.tile([D, NT], f32, tag="op", name="op")
            nc.tensor.matmul(op[:], w2[:, 0:128], r_lo[:], start=True, stop=False)
            nc.tensor.matmul(op[:], w2[:, 128:256], r_hi[:], start=False, stop=True)

            osb = outp_p.tile([D, NT], f32, tag="osb", name="osb")
            nc.scalar.activation(osb[:], op[:], AF.Identity, bias=bc[:, 4:5])
            nc.sync.dma_start(out[:, t0:t0 + NT], osb[:])
        ctx.close()

    _legalize_multi_waits(nc)
    return nc


def _prep_host(x, presence, idx_i, idx_j,
               ops_W1, ops_b1, ops_W2, ops_b2,
               pres_W1, pres_b1, pres_W2, pres_b2,
               fus_W1, fus_b1, fus_W2, fus_b2):
    x = np.asarray(x, np.float32)
    ops_W1 = np.asarray(ops_W1, np.float32)
    ops_b1 = np.asarray(ops_b1, np.float32)
    ops_W2 = np.asarray(ops_W2, np.float32)
    ops_b2 = np.asarray(ops_b2, np.float32)
    fus_W1 = np.asarray(fus_W1, np.float32)
    fus_b1 = np.asarray(fus_b1, np.float32)
    fus_W2 = np.asarray(fus_W2, np.float32)
    fus_b2 = np.asarray(fus_b2, np.float32)
    pres_W1 = np.asarray(pres_W1, np.float32)
    pres_b1 = np.asarray(pres_b1, np.float32)
    pres_W2 = np.asarray(pres_W2, np.float32)
    pres_b2 = np.asarray(pres_b2, np.float32)

    # per-core x panels, cols (c-major, b-inner)
    xTs, xns, xrs = [], [], []
    norms = np.linalg.norm(x, axis=2)                    # [B, NCOLS]
    xn_full = x / (norms[:, :, None] + EPS)
    xr_full = 1.0 / (x + EPS)
    for k in range(NCORES):
        bs = slice(k * BC, (k + 1) * BC)
        # [BC, NCOLS, D] -> [D, NCOLS, BC] -> [D, NCOLS*BC]
        def panel(a):
            return np.ascontiguousarray(
                np.transpose(a[bs], (2, 1, 0)).reshape(D, NCOL_DEV)).astype(BF)
        xTs.append(panel(x))
        xns.append(panel(xn_full))
        xrs.append(panel(xr_full))

    # one-hot rows [ones, t0, t1, t2, t3] per core; token = p*BC + b_local
    pa = np.asarray(presence)[:, idx_i].astype(np.int64)   # [B, P]
    pb = np.asarray(presence)[:, idx_j].astype(np.int64)
    tsel = 2 * pa + pb
    ohs = []
    for k in range(NCORES):
        bs = slice(k * BC, (k + 1) * BC)
        tloc = np.transpose(tsel[bs], (1, 0)).reshape(NTOK)  # p-major, b-inner
        o = np.zeros((5, NTOK), np.float32)
        o[0] = 1.0
        for tt in range(4):
            o[1 + tt] = (tloc == tt)
        ohs.append(o.astype(BF))

    # stage-1 weights: mult, cos, abs, divab, divba, add, sub, subba(-W1_4)
    Wcat = np.concatenate([
        ops_W1[0], ops_W1[2], ops_W1[3], ops_W1[5],
        ops_W1[5], ops_W1[1], ops_W1[4], -ops_W1[4]], axis=1)  # [128, 512]

    # chunk h-order: [mult,cos | absd,divab | divba,add | sub,subba]
    feat_fusblk = [0, 2, 3, 6, 7, 1, 4, 5]   # fus_W1 block per h slot
    feat_w2 = [MLP_IDX[i] for i in feat_fusblk]
    Mf = np.zeros((512, 256), np.float32)
    c0 = fus_b1.copy()
    for s in range(8):
        blk = fus_W1[feat_fusblk[s] * D:(feat_fusblk[s] + 1) * D]  # [128,256]
        Mf[s * 64:(s + 1) * 64] = ops_W2[feat_w2[s]] @ blk
        c0 += ops_b2[feat_w2[s]] @ blk
    # device layout [128, 4*256]: block c at cols [c*256, (c+1)*256)
    Mf_dev = np.zeros((128, 4 * 256), np.float32)
    for c in range(4):
        Mf_dev[:, c * 256:(c + 1) * 256] = Mf[c * 128:(c + 1) * 128]

    # presence table + c0; rows [c0, t0..t3] matching onehot rows
    fus_blk_p = fus_W1[8 * D:]
    Mp = pres_W2 @ fus_blk_p
    ptabc = np.zeros((5, 256), np.float32)
    ptabc[0] = c0 + pres_b2 @ fus_blk_p
    for tt in range(4):
        hp = np.maximum(pres_W1[tt] + pres_b1, 0.0)
        ptabc[1 + tt] = hp @ Mp

    # chunk relu biases: [b1_mult;b1_cos], [b1_abs;b1_divab],
    #                    [b1_divba;b1_add], [b1_sub;b1_subba]
    bcat = np.stack([
        np.concatenate([ops_b1[0], ops_b1[2]]),
        np.concatenate([ops_b1[3], ops_b1[5]]),
        np.concatenate([ops_b1[5], ops_b1[1]]),
        np.concatenate([ops_b1[4], ops_b1[4]]),
        fus_b2,
    ], axis=1).astype(np.float32)                         # [128, 5]

    w2_dev = np.concatenate([fus_W2[0:128], fus_W2[128:256]], axis=1)  # [128,256]

    shared = dict(
        w1=np.ascontiguousarray(Wcat).astype(BF),
        mf=np.ascontiguousarray(Mf_dev).astype(BF),
        pt=np.ascontiguousarray(ptabc).astype(BF),
        w2=np.ascontiguousarray(w2_dev).astype(BF),
        bc=np.ascontiguousarray(bcat),
    )
    in_maps = []
    for k in range(NCORES):
        m = dict(shared)
        m["xT"] = xTs[k]
        m["xn"] = xns[k]
        m["xr"] = xrs[k]
        m["oh"] = ohs[k]
        in_maps.append(m)
    return in_maps


def kernel(x, presence, idx_i, idx_j,
           ops_W1, ops_b1, ops_W2, ops_b2,
           pres_W1, pres_b1, pres_W2, pres_b2,
           fus_W1, fus_b1, fus_W2, fus_b2):
    idx_i = np.asarray(idx_i)
    idx_j = np.asarray(idx_j)
    key = (idx_i.tobytes(), idx_j.tobytes())
    if key not in _CACHE:
        _CACHE.clear()
        _CACHE[key] = _build_program(_runs(idx_i, idx_j))
    nc = _CACHE[key]

    in_maps = _prep_host(x, presence, idx_i, idx_j,
                         ops_W1, ops_b1, ops_W2, ops_b2,
                         pres_W1, pres_b1, pres_W2, pres_b2,
                         fus_W1, fus_b1, fus_W2, fus_b2)
    res = run_bass_kernel_spmd(nc, in_maps, core_ids=list(range(NCORES)))
    kernel._last_results = res

    # reassemble: out_full[b, p, d] = core_out[b//BC][d, p*BC + b%BC]
    out = np.empty((B, P_TOT, D), np.float32)
    for k in range(NCORES):
        co = res.results[k]["out"]                  # [D, NTOK]
        co = co.reshape(D, P_TOT, BC)               # [d, p, b_local]
        out[k * BC:(k + 1) * BC] = np.transpose(co, (2, 1, 0))
    return out.astype(np.asarray(x).dtype)
